# revision 1
# baseline (speedup 1.0000x reference)
"""Linear-CKA map kernel for Trainium2 (8 NeuronCores, SPMD, no collectives).

Math: for activations X[l] ([B, D] per layer), the reference computes
Gram matrices G_l = X_l X_l^T, double-centers them (Gc = H G H), and
hsic[i,j] = <Gc_i, Gc_j>, cka = hsic / sqrt(diag outer).

We use the expansion (H idempotent, G symmetric):
    hsic_ij = S_ij - (2/B) * T_ij + u_i u_j / B^2
      S_ij = <G_i, G_j>
      rowsum_l[b] = sum_c G_l[b, c] = X_l[b, :] . s_l,  s_l = sum_b X_l[b, :]
      T_ij = sum_b rowsum_i[b] rowsum_j[b]
      u_l  = s_l . s_l

Sharding: the Gram is symmetric, so only its block upper triangle is
needed.  With 16x16 blocks of [128, 128], core k computes the cyclic
cover blocks (bi, (bi + t) mod 16), t = 0..8, for its two block rows
bi in {2k, 2k+1} -- a perfectly uniform SPMD program (18 blocks per
core; every unordered block pair is covered once, except the t=8
antipodal blocks which two cores split).  Weight classes: t=0 diag
blocks count once, t=1..7 twice, t=8 once; the three classes accumulate
into separate [24,24] PSUM groups and the host combines g0 + 2*g1 + g2.

Per-core input: xr = X^T columns [2k*128 : 2k*128+1280] (mod B), in
fp8-e4m3 (CKA is a normalized statistic, so input quantization noise
stays ~1e-4 in the output) -- both Gram matmul operands come from this
one 63 MB slice, read exactly once, and the matmuls run in DoubleRow
mode (256-deep contraction, 2 fp8 MACs/cell/cycle).  S reduces in two
overlapped pieces: pairs within the first LA layers stream through the
otherwise-idle VectorE *during* the DMA-bound main loop (they unlock
as each layer's Gram lands), while TensorE finishes the rest in a short
tail: the PSUM->SBUF copies interleave layers ([b, c, layer] bf16),
then [128b, 4x24] x [128b, 4x14] matmuls (4 Gram columns per call, rhs
restricted to the remaining LB layers) accumulate S in PSUM; the host
keeps the diagonal blocks.  Partial S is the only device output,
summed on the host -- no device collective at all.  The O(L*B*D)
row-sum statistics T and u are computed on the host (0.02% of FLOPs).
"""

import numpy as np
import ml_dtypes

L, B, D = 24, 2048, 2048
NCORES = 8
P = 128
NBLK = B // P               # 16 block rows/cols
JT = D // (2 * P)           # 8 double-row contraction tiles (256 deep each)
JG = 2                      # j-tiles fetched per rhs DMA
NT = 9                      # cyclic block offsets t = 0..8 per block row
NR = 2                      # block rows per core
NXC = NT + NR - 1           # 10 column chunks staged per core
WC = NT * P                 # 1152 Gram columns per block row
LA = 10                     # layers whose intra-pairs reduce on idle VectorE
LB = L - LA                 # layers handled by the TensorE S-stage rhs
NPA = LA * (LA + 1) // 2    # VectorE pair count

_NC_CACHE = {}


def _build():
    if "nc" in _NC_CACHE:
        return _NC_CACHE["nc"]
    import concourse.bass as bass
    from concourse import bacc, mybir, tile

    f32 = mybir.dt.float32
    bf16 = mybir.dt.bfloat16
    fp8 = mybir.dt.float8e4
    DR = mybir.MatmulPerfMode.DoubleRow
    Act = mybir.ActivationFunctionType
    Alu = mybir.AluOpType

    nc = bacc.Bacc("TRN2", target_bir_lowering=False, debug=False)

    # xr is host-packed to exactly match the SBUF staging tiles: one fully
    # linear [P, JG, 2, 1280] block per (layer, jg) DMA
    xr = nc.dram_tensor(
        "xr", [L, JT // JG, P, JG, 2, NXC * P], fp8, kind="ExternalInput"
    )
    s_out = nc.dram_tensor("s_out", [3, 4 * L, 4 * LB], f32, kind="ExternalOutput")
    a_out = nc.dram_tensor("a_out", [1, NR * 3 * NPA], f32, kind="ExternalOutput")

    with tile.TileContext(nc) as tc:
        with (
            tc.tile_pool(name="gb", bufs=1) as gbpool,
            tc.tile_pool(name="rt", bufs=5) as rtpool,
            tc.tile_pool(name="small", bufs=1) as smallpool,
            tc.tile_pool(name="psum", bufs=2, space=bass.MemorySpace.PSUM) as psumpool,
            tc.tile_pool(name="psumS", bufs=1, space=bass.MemorySpace.PSUM) as psSpool,
        ):
            # persistent SBUF: interleaved Gram store [b, c, layer] per row
            gbig16 = [
                gbpool.tile([P, P, L], bf16, tag=f"Gb16{r}", name=f"Gb16{r}")
                for r in range(NR)
            ]
            gbig8 = [
                gbpool.tile([P, WC - P, L], fp8, tag=f"Gb8{r}", name=f"Gb8{r}")
                for r in range(NR)
            ]
            # NOTE: matmul start=True pending-zeroes its whole PSUM bank, so
            # each bank may host only ONE accumulation group at a time (the
            # 512/512/128 Gram split below is bank-aligned for this reason,
            # and the S classes accumulate sequentially with copies between).
            QW = 4 * L          # S-stage stationary width (4 Gram columns)
            QN = 4 * LB         # S-stage moving width (B layers only)
            ptS = psSpool.tile([P, 3 * QN], f32, tag="psS")
            # VectorE A-block pair accumulators, (r, class)-sliced
            pairacc = smallpool.tile([P, NR * 3 * NPA], f32, tag="pairacc")
            sttout = smallpool.tile([P, WC], bf16, tag="sttout")

            for l in range(L):
                pt = [
                    psumpool.tile([P, WC], f32, tag="pm", name=f"pm{r}")
                    for r in range(NR)
                ]
                for jg in range(JT // JG):
                    rt = rtpool.tile([P, JG, 2, NXC * P], fp8, tag="rt")
                    nc.sync.dma_start(rt[:, :, :, :], xr[l, jg])
                    for jj in range(JG):
                        j = jg * JG + jj
                        for r in range(NR):
                            lhs = rt[:, jj, :, r * P : (r + 1) * P]
                            # 9 cyclic blocks = contiguous 1152 rhs columns,
                            # split 512/512/128 on PSUM bank boundaries
                            for c0, cw in ((0, 512), (512, 512), (1024, 128)):
                                nc.tensor.matmul(
                                    pt[r][:, c0 : c0 + cw],
                                    lhsT=lhs,
                                    rhs=rt[:, jj, :, r * P + c0 : r * P + c0 + cw],
                                    start=(j == 0),
                                    stop=(j == JT - 1),
                                    perf_mode=DR,
                                )
                if l == L - 1:
                    # issue both small diag-block copies first so the g0
                    # quads unlock while the big scaled fp8 copies stream
                    # (VectorE is still draining its pair backlog here)
                    nc.scalar.copy(gbig16[0][:, :, l], pt[0][:, 0:P])
                    nc.scalar.copy(gbig16[1][:, :, l], pt[1][:, 0:P])
                    nc.scalar.mul(gbig8[0][:, :, l], pt[0][:, P:WC], 0.5)
                    nc.scalar.mul(gbig8[1][:, :, l], pt[1][:, P:WC], 0.5)
                else:
                    for r in range(NR):
                        nc.scalar.copy(gbig16[r][:, :, l], pt[r][:, 0:P])
                        nc.scalar.mul(gbig8[r][:, :, l], pt[r][:, P:WC], 0.5)
                # A-block pairs (i, l), i <= l < LA reduce on the otherwise
                # idle VectorE while the DMA-bound main loop continues; the
                # three weight classes accumulate into separate columns
                if l < LA:
                    for i in range(l + 1):
                        p = l * (l + 1) // 2 + i
                        for r in range(NR):
                            for cls, (st, lo, hi) in enumerate(
                                ((0, 0, P), (1, 0, 7 * P), (1, 7 * P, 8 * P))
                            ):
                                gsrc = gbig16[r] if st == 0 else gbig8[r]
                                nc.vector.scalar_tensor_tensor(
                                    out=sttout[:, lo:hi],
                                    in0=gsrc[:, lo:hi, i],
                                    scalar=1.0,
                                    in1=gsrc[:, lo:hi, l],
                                    op0=Alu.mult,
                                    op1=Alu.mult,
                                    accum_out=pairacc[
                                        :, (r * 3 + cls) * NPA + p : (r * 3 + cls) * NPA + p + 1
                                    ],
                                )

            # partition-reduce the VectorE pair accumulators on idle GpSimd
            asum = smallpool.tile([P, NR * 3 * NPA], f32, tag="asum")
            nc.gpsimd.tensor_reduce(
                asum[0:1, :], pairacc[:, :], axis=mybir.AxisListType.C, op=Alu.add
            )
            nc.sync.dma_start(a_out[:, :], asum[0:1, :])

            # S-stage on TensorE: [128b, 96] x [128b, 96] matmuls, 4 Gram
            # columns per call (gbig's [c, l] layout is contiguous, so 4
            # columns = one flat 96-wide operand).  Only the 4 diagonal
            # [24, 24] blocks of each [96, 96] product matter; the host
            # discards the cross-column junk.  Weight classes accumulate
            # SEQUENTIALLY (one live group in the shared PSUM bank) and are
            # copied out before the next class's start=True re-marks the bank.
            # lhsT spans all L layers (out rows cover every i), rhs spans only
            # the LB "B" layers -- the A-block intra-pairs came from VectorE
            sall = smallpool.tile([P, 3 * QN], f32, tag="sall")
            # class 0: bf16 quads on the diag-block store; classes 1/2:
            # fp8 DoubleRow octs (pair dim = c-offset 0..3 vs 4..7) on the
            # off-diag store -- the pair dim contracts away so the [96, 56]
            # extraction is identical
            cls_src = {0: (0, 0, P, 4), 1: (1, 0, 7 * P, 8), 2: (1, 7 * P, 8 * P, 8)}
            for cls in range(3):
                st, lo, hi, step = cls_src[cls]
                for r in range(NR):
                    for c in range(lo, hi, step):
                        if st == 0:
                            nc.tensor.matmul(
                                ptS[0:QW, cls * QN : (cls + 1) * QN],
                                lhsT=gbig16[r][:, c : c + 4, :],
                                rhs=gbig16[r][:, c : c + 4, LA:L],
                                start=(r == 0 and c == lo),
                                stop=(r == NR - 1 and c == hi - step),
                            )
                        else:
                            nc.tensor.matmul(
                                ptS[0:QW, cls * QN : (cls + 1) * QN],
                                lhsT=gbig8[r][:, c : c + 8, :].rearrange(
                                    "p (i x) l -> p i (x l)", i=2
                                ),
                                rhs=gbig8[r][:, c : c + 8, LA:L].rearrange(
                                    "p (i x) l -> p i x l", i=2
                                ),
                                start=(r == 0 and c == lo),
                                stop=(r == NR - 1 and c == hi - step),
                                perf_mode=DR,
                            )
                nc.scalar.copy(
                    sall[0:QW, cls * QN : (cls + 1) * QN],
                    ptS[0:QW, cls * QN : (cls + 1) * QN],
                )
                nc.sync.dma_start(
                    s_out[cls], sall[0:QW, cls * QN : (cls + 1) * QN]
                )

    nc.compile()
    _NC_CACHE["nc"] = nc
    return nc


def _run(activations, trace=False):
    from concourse.bass_utils import run_bass_kernel_spmd

    x = np.asarray(activations, dtype=np.float32)
    assert x.shape == (L, B, D)
    xt_np = np.ascontiguousarray(x.transpose(0, 2, 1)).astype(ml_dtypes.float8_e4m3)
    s_star = xt_np.astype(np.float64).sum(axis=2)  # [L, D], exact sum of fp8 X

    in_maps = []
    for c in range(NCORES):
        lo = NR * c * P
        rolled = np.concatenate([xt_np[:, :, lo:], xt_np[:, :, :lo]], axis=2)[
            :, :, : NXC * P
        ]
        # pack to the SBUF staging layout: [L, jg, p, jj, i, n] so each
        # (layer, jg) DMA is one fully contiguous block
        packed = np.ascontiguousarray(
            rolled.reshape(L, JT // JG, JG, 2, P, NXC * P).transpose(0, 1, 4, 2, 3, 5)
        )
        in_maps.append({"xr": packed})
    nc = _build()
    try:
        res = run_bass_kernel_spmd(
            nc, in_maps, core_ids=list(range(NCORES)), trace=trace
        )
    except Exception:
        # transient NRT_EXEC_UNIT_UNRECOVERABLE device states have been
        # observed to clear on the next attempt
        import time

        time.sleep(5)
        res = run_bass_kernel_spmd(
            nc, in_maps, core_ids=list(range(NCORES)), trace=trace
        )

    S = np.zeros((L, L), dtype=np.float64)
    for c in range(NCORES):
        # TensorE part: [3, 4*L, 4*LB] quad blocks, diagonal-in-quad only
        g = res.results[c]["s_out"].astype(np.float64).reshape(3, 4, L, 4, LB)
        gd = [sum(g[i, d, :, d, :] for d in range(4)) for i in range(3)]
        Sc = np.zeros((L, L))
        Sc[:, LA:] = gd[0] + 8.0 * gd[1] + 4.0 * gd[2]
        Sc[LA:, :LA] = Sc[:LA, LA:].T
        # VectorE part: A-block pairs, (r, class)-sliced partials
        a = res.results[c]["a_out"].astype(np.float64).reshape(NR, 3, NPA)
        av = a.sum(axis=0)
        pa = av[0] + 8.0 * av[1] + 4.0 * av[2]
        for l in range(LA):
            for i in range(l + 1):
                v = pa[l * (l + 1) // 2 + i]
                Sc[i, l] = v
                Sc[l, i] = v
        S += Sc

    # row-sum statistics are O(L*B*D) -- computed host-side on the same
    # quantized values the device consumed
    xq = xt_np.astype(np.float32)                  # [L, D, B]
    rowsum = np.einsum("ldb,ld->lb", xq, s_star.astype(np.float32))
    T = np.einsum("ib,jb->ij", rowsum, rowsum, dtype=np.float64)
    u = np.einsum("ld,ld->l", s_star, s_star)
    hsic = S - (2.0 / B) * T + np.outer(u, u) / (B * B)
    norms = np.sqrt(np.diagonal(hsic))
    cka = hsic / (norms[:, None] * norms[None, :])
    return cka.astype(np.float32), res


def kernel(activations):
    cka, _ = _run(activations, trace=False)
    return cka


def run_traced(activations):
    return _run(activations, trace=True)



# revision 6
# speedup vs baseline: 1.0478x; 1.0478x over previous
"""Linear-CKA map kernel for Trainium2 (8 NeuronCores, SPMD, no collectives).

Math: for activations X[l] ([B, D] per layer), the reference computes
Gram matrices G_l = X_l X_l^T, double-centers them (Gc = H G H), and
hsic[i,j] = <Gc_i, Gc_j>, cka = hsic / sqrt(diag outer).

We use the expansion (H idempotent, G symmetric):
    hsic_ij = S_ij - (2/B) * T_ij + u_i u_j / B^2
      S_ij = <G_i, G_j>
      rowsum_l[b] = X_l[b, :] . s_l,  s_l = sum_b X_l[b, :]
      T_ij = sum_b rowsum_i[b] rowsum_j[b],  u_l = s_l . s_l
The O(L*B*D) statistics T and u are computed on the host (0.02% of FLOPs).

Sharding (shift-by-1 cyclic cover): B is split into 16 row-chunks of 128.
Core k loads the 8 chunks {0,1,3,7,8,9,11,15} + k (mod 16) -- 4 antipodal
pairs whose mod-8 offsets {0,1,3,7} realize every difference class.  The
fixed 17-block schedule (2 diag blocks (7,7),(15,15) + 15 off-diag pairs),
shifted by +k per core, covers every unordered chunk pair of the Gram
exactly once and every diagonal chunk exactly once -- an exact cover, so
the host combines partial S sums with just two class weights (diag 1,
off-diag 8 = 2 for pair symmetry x 4 for the fp8 0.5^2 storage scale).
Per-core input drops from 10/16 to 8/16 of X (50.3 MB fp8), read once.

The 17 blocks pack into 5 matmul runs (one lhs chunk, <=4 consecutive rhs
chunks in SBUF order [9,8,0,1,15,7,3,11]), each accumulating in its own
2 KB PSUM bank (512 f32) over 8 DoubleRow 256-deep fp8 contraction steps.
Gram blocks land in SBUF as bf16 (diag) / fp8 x0.5 (off-diag), layer-
interleaved [b, c, layer].  S then reduces on TensorE incrementally: after
every 8th layer, [128b, 4c x l<=8g+8] x [128b, 4c x 8] matmuls accumulate
<G_i, G_j> for j in that layer group and all i <= group end (the lhsT only
spans layers already written, so groups 0/1 overlap the DMA-bound main
loop; only group 2 is tail).  Host mirrors the upper wedge to complete S.
"""

import numpy as np
import ml_dtypes

L, B, D = 24, 2048, 2048
NCORES = 8
P = 128
JT = 8                  # 256-deep DoubleRow contraction steps
JG = 2                  # j-tiles fetched per rhs DMA
NX = 8                  # chunks staged per core
SB_W = NX * P           # 1024 staged Gram-row columns
NW2 = 15                # off-diag (weight-2) blocks
SGRP = 8                # layers per S-stage group
NGRP = L // SGRP

# SBUF position -> chunk offset (mod 16); core k stages chunks ORDER + k
ORDER = [9, 8, 0, 1, 15, 7, 3, 11]
# Gram matmul runs: (psum tag, psum col0, lhs pos, rhs pos0, width in chunks)
# blocks, in psum-col order:
#   P1[0:512]    = (15,8),(15,0),(15,1),(15,15)d
#   P1[512:896]  = (7,9),(7,8),(7,0)
#   P2[0:384]    = (7,15),(7,7)d,(7,3)
#   P2[512:896]  = (3,9),(3,8),(3,0)
#   P2[1024:1536]= (11,8),(11,0),(11,1),(11,15)
RUNS = [
    ("P1", 0,    4, 1, 4),
    ("P1", 512,  5, 0, 3),
    ("P2", 0,    5, 4, 3),
    ("P2", 512,  6, 0, 3),
    ("P2", 1024, 7, 1, 4),
]
# PSUM -> SBUF copies: (tag, col0, dst, dst col, width); dst 0/1 = gbig16, 2 = gbig8
COPIES = [
    ("P1", 0,    2, 0,    384),
    ("P1", 384,  0, 0,    128),
    ("P1", 512,  2, 384,  384),
    ("P2", 0,    2, 768,  128),
    ("P2", 128,  1, 0,    128),
    ("P2", 256,  2, 896,  128),
    ("P2", 512,  2, 1024, 384),
    ("P2", 1024, 2, 1408, 512),
]

_NC_CACHE = {}


def _build():
    if "nc" in _NC_CACHE:
        return _NC_CACHE["nc"]
    import concourse.bass as bass
    from concourse import bacc, mybir, tile

    f32 = mybir.dt.float32
    bf16 = mybir.dt.bfloat16
    fp8 = mybir.dt.float8e4
    DR = mybir.MatmulPerfMode.DoubleRow

    nc = bacc.Bacc("TRN2", target_bir_lowering=False, debug=False)

    # host-packed to match the SBUF staging tiles: one fully linear
    # [P, JG, 2, 1024] block per (layer, jg) DMA
    xr = nc.dram_tensor(
        "xr", [L, JT // JG, P, JG, 2, SB_W], fp8, kind="ExternalInput"
    )
    s_out = nc.dram_tensor("s_out", [NGRP, L, 16], f32, kind="ExternalOutput")

    with tile.TileContext(nc) as tc:
        with (
            tc.tile_pool(name="gb", bufs=1) as gbpool,
            tc.tile_pool(name="rt", bufs=5) as rtpool,
            tc.tile_pool(name="small", bufs=2) as smallpool,
            tc.tile_pool(name="psum", bufs=1, space=bass.MemorySpace.PSUM) as psumpool,
            tc.tile_pool(name="psumS", bufs=1, space=bass.MemorySpace.PSUM) as psSpool,
        ):
            # persistent SBUF Gram store, layer-interleaved [b, c, layer]
            gbig16 = [
                gbpool.tile([P, P, L], bf16, tag=f"Gb16{r}", name=f"Gb16{r}")
                for r in range(2)
            ]
            gbig8 = gbpool.tile([P, NW2 * P, L], fp8, tag="Gb8", name="Gb8")

            for l in range(L):
                # 5 Gram accumulation runs, one per 2 KB PSUM bank
                pt1 = psumpool.tile([P, 1024], f32, tag="P1", name="pt1")
                pt2 = psumpool.tile([P, 1536], f32, tag="P2", name="pt2")
                pts = {"P1": pt1, "P2": pt2}
                for jg in range(JT // JG):
                    rt = rtpool.tile([P, JG, 2, SB_W], fp8, tag="rt", name="rt")
                    nc.sync.dma_start(rt[:, :, :, :], xr[l, jg])
                    for jj in range(JG):
                        j = jg * JG + jj
                        for (tag, c0, lp, rp, w) in RUNS:
                            nc.tensor.matmul(
                                pts[tag][:, c0 : c0 + P * w],
                                lhsT=rt[:, jj, :, lp * P : (lp + 1) * P],
                                rhs=rt[:, jj, :, rp * P : (rp + w) * P],
                                start=(j == 0),
                                stop=(j == JT - 1),
                                perf_mode=DR,
                            )
                for (tag, c0, dst, dc, w) in COPIES:
                    src = pts[tag][:, c0 : c0 + w]
                    if dst == 2:
                        nc.scalar.mul(gbig8[:, dc : dc + w, l], src, 0.5)
                    else:
                        nc.scalar.copy(gbig16[dst][:, :, l], src)

                # S-stage group: after layers [8g, 8g+8) land, reduce
                # <G_i, G_j> for j in the group, i <= 8g+7 (operands only
                # span written layers, so groups 0/1 overlap the main loop).
                # The stationary operand must be a single-free-dim AP, so S
                # accumulates per Gram column (bf16) / column pair (fp8 DR);
                # PSUM then holds S[i, j] directly -- no quad extraction.
                if l % SGRP == SGRP - 1:
                    g = l // SGRP
                    n = SGRP * (g + 1)          # lhs layers available
                    ptS = psSpool.tile([P, 1024], f32, tag="psS", name="ptS")
                    for r in range(2):
                        for c in range(P):
                            nc.tensor.matmul(
                                ptS[0:n, 0:8],
                                lhsT=gbig16[r][:, c, 0:n],
                                rhs=gbig16[r][:, c, SGRP * g : SGRP * (g + 1)],
                                start=(r == 0 and c == 0),
                                stop=(r == 1 and c == P - 1),
                            )
                    for c0 in range(0, NW2 * P, 8):
                        a4 = gbig8[:, c0 : c0 + 8, 0:n].rearrange(
                            "p (i x) l -> p i x l", i=2
                        )
                        b4 = gbig8[
                            :, c0 : c0 + 8, SGRP * g : SGRP * (g + 1)
                        ].rearrange("p (i x) l -> p i x l", i=2)
                        for x in range(4):
                            nc.tensor.matmul(
                                ptS[0:n, 512:520],
                                lhsT=a4[:, :, x, :],
                                rhs=b4[:, :, x, :],
                                start=(c0 == 0 and x == 0),
                                stop=(c0 == NW2 * P - 8 and x == 3),
                                perf_mode=DR,
                            )
                    sall = smallpool.tile([P, 16], f32, tag="sall", name="sall")
                    nc.scalar.copy(sall[0:n, 0:8], ptS[0:n, 0:8])
                    nc.scalar.copy(sall[0:n, 8:16], ptS[0:n, 512:520])
                    nc.sync.dma_start(s_out[g, 0:n, :], sall[0:n, :])

    nc.compile()
    _NC_CACHE["nc"] = nc
    return nc


def _pack_inputs(x):
    xt_np = np.ascontiguousarray(x.transpose(0, 2, 1)).astype(ml_dtypes.float8_e4m3)
    in_maps = []
    for k in range(NCORES):
        chunk_ids = [(o + k) % 16 for o in ORDER]
        cols = np.concatenate(
            [np.arange(c * P, (c + 1) * P) for c in chunk_ids]
        )
        sel = xt_np[:, :, cols]  # [L, D, 1024]
        packed = np.ascontiguousarray(
            sel.reshape(L, JT // JG, JG, 2, P, SB_W).transpose(0, 1, 4, 2, 3, 5)
        )
        in_maps.append({"xr": packed})
    return xt_np, in_maps


def _run(activations, trace=False):
    from concourse.bass_utils import run_bass_kernel_spmd

    x = np.asarray(activations, dtype=np.float32)
    assert x.shape == (L, B, D)
    xt_np, in_maps = _pack_inputs(x)

    nc = _build()
    try:
        res = run_bass_kernel_spmd(
            nc, in_maps, core_ids=list(range(NCORES)), trace=trace
        )
    except Exception:
        # transient NRT_EXEC_UNIT_UNRECOVERABLE device states have been
        # observed to clear on the next attempt
        import time

        time.sleep(5)
        res = run_bass_kernel_spmd(
            nc, in_maps, core_ids=list(range(NCORES)), trace=trace
        )

    # accumulate the wedge S[i, j] for i <= group_end(j); mirror the rest
    S = np.zeros((L, L), dtype=np.float64)
    for k in range(NCORES):
        g = res.results[k]["s_out"].astype(np.float64)  # [NGRP, L, 16]
        for grp in range(NGRP):
            n = SGRP * (grp + 1)
            S[:n, SGRP * grp : SGRP * (grp + 1)] += (
                g[grp, :n, 0:8] + 8.0 * g[grp, :n, 8:16]
            )
    for i in range(L):
        for j in range(L):
            if i > SGRP * (j // SGRP + 1) - 1:
                S[i, j] = S[j, i]

    # row-sum statistics are O(L*B*D) -- computed host-side on the same
    # quantized values the device consumed
    s_star = xt_np.astype(np.float64).sum(axis=2)  # [L, D]
    xq = xt_np.astype(np.float32)                  # [L, D, B]
    rowsum = np.einsum("ldb,ld->lb", xq, s_star.astype(np.float32))
    T = np.einsum("ib,jb->ij", rowsum, rowsum, dtype=np.float64)
    u = np.einsum("ld,ld->l", s_star, s_star)
    hsic = S - (2.0 / B) * T + np.outer(u, u) / (B * B)
    norms = np.sqrt(np.diagonal(hsic))
    cka = hsic / (norms[:, None] * norms[None, :])
    return cka.astype(np.float32), res


def kernel(activations):
    cka, _ = _run(activations, trace=False)
    return cka


def run_traced(activations):
    return _run(activations, trace=True)


# revision 14
# speedup vs baseline: 1.0994x; 1.0493x over previous
"""Linear-CKA map kernel for Trainium2 (8 NeuronCores, SPMD, no collectives).

Math: for activations X[l] ([B, D] per layer), the reference computes
Gram matrices G_l = X_l X_l^T, double-centers them (Gc = H G H), and
hsic[i,j] = <Gc_i, Gc_j>, cka = hsic / sqrt(diag outer).

We use the expansion (H idempotent, G symmetric):
    hsic_ij = S_ij - (2/B) * T_ij + u_i u_j / B^2
      S_ij = <G_i, G_j>
      rowsum_l[b] = X_l[b, :] . s_l,  s_l = sum_b X_l[b, :]
      T_ij = sum_b rowsum_i[b] rowsum_j[b],  u_l = s_l . s_l
The O(L*B*D) statistics T and u are computed on the host (0.02% of FLOPs).

Sharding (shift-by-1 cyclic cover): B is split into 16 row-chunks of 128.
Core k loads the 8 chunks {0,1,3,7,8,9,11,15} + k (mod 16) -- 4 antipodal
pairs whose mod-8 offsets {0,1,3,7} realize every difference class.  The
fixed 17-block schedule (diag blocks (7,7),(15,15) + 15 off-diag pairs),
shifted by +k per core, covers every unordered chunk pair of the Gram
exactly once and every diagonal chunk exactly once -- an exact cover, so
the host combines partial S sums with two class weights (diag 1, off-diag
8 = 2 for pair symmetry x 4 for the fp8 0.5^2 storage scale).  Per-core
input is 8/16 of X (50.3 MB fp8), read exactly once; in the cost model the
DMA stream is the ~140 us roofline and everything else must hide under it.

The 17 blocks pack into 5 matmul runs (one lhs chunk, <=4 consecutive rhs
chunks in SBUF order [9,8,0,1,15,7,3,11]), each accumulating in its own
2 KB PSUM bank over 8 DoubleRow 256-deep fp8 contraction steps.  P1 (2
banks) is double-buffered so its copies leave the critical path; P2's
runs and copies are emitted first to shorten the WAR chain into the next
layer.  Gram blocks land in SBUF as bf16 (diag) / fp8 x0.5 (off-diag),
layer-interleaved [b, c, layer], copies split across the Scalar and
Vector engines.  S = <G_i, G_j> reduces on TensorE per Gram column (bf16)
/ DoubleRow column pair (fp8): rhs layer-groups (0-7, 8-15, 16-19, 20-23)
with lhsT spanning only written layers; each group's ~1.2k instructions
are SPREAD over the following layers' slack (a group emitted as one burst
occupies PE, stalls Gram consumption, and starves the DMA queue for ~4
us), leaving only the last 4-layer group as tail.  Host mirrors the
wedge S[i, j] (present for i < group_hi(j)) to complete S.
"""

import numpy as np
import ml_dtypes

L, B, D = 24, 2048, 2048
NCORES = 8
P = 128
JT = 8                  # 256-deep DoubleRow contraction steps
JG = 2                  # j-tiles fetched per rhs DMA
NX = 8                  # chunks staged per core
SB_W = NX * P           # 1024 staged Gram-row columns
NW2 = 15                # off-diag (weight-2) blocks

# S-stage rhs layer groups: (rhs lo, rhs hi, spread-over layers [lo, hi)).
# Spread windows must not overlap: there is a single ptS accumulator bank.
SGROUPS = [(0, 8, 8, 16), (8, 16, 16, 20), (16, 20, 20, 23), (20, 24, 24, 24)]

# SBUF position -> chunk offset (mod 16); core k stages chunks ORDER + k
ORDER = [9, 8, 0, 1, 15, 7, 3, 11]
# Gram matmul runs: (psum tag, psum col0, lhs pos, rhs pos0, width in chunks)
# blocks, in psum-col order:
#   P2[0:384]    = (7,15),(7,7)d,(7,3)
#   P2[512:896]  = (3,9),(3,8),(3,0)
#   P2[1024:1536]= (11,8),(11,0),(11,1),(11,15)
#   P1[0:512]    = (15,8),(15,0),(15,1),(15,15)d
#   P1[512:896]  = (7,9),(7,8),(7,0)
RUNS = [
    ("P2", 0,    5, 4, 3),
    ("P2", 512,  6, 0, 3),
    ("P2", 1024, 7, 1, 4),
    ("P1", 0,    4, 1, 4),
    ("P1", 512,  5, 0, 3),
]
# PSUM -> SBUF copies: (tag, col0, dst, dst col, width, engine);
# dst 0/1 = gbig16, 2 = gbig8; engine 0 = scalar (Act), 1 = vector (DVE)
COPIES = [
    ("P2", 0,    2, 768,  128, 0),
    ("P2", 128,  1, 0,    128, 0),
    ("P2", 256,  2, 896,  128, 0),
    ("P2", 512,  2, 1024, 384, 1),
    ("P2", 1024, 2, 1408, 512, 0),
    ("P1", 0,    2, 0,    384, 1),
    ("P1", 384,  0, 0,    128, 0),
    ("P1", 512,  2, 384,  384, 1),
]

_NC_CACHE = {}


def _build(no_s=False, no_mm=False, no_copy=False):
    key = ("nc", no_s, no_mm, no_copy)
    if key in _NC_CACHE:
        return _NC_CACHE[key]
    import concourse.bass as bass
    from concourse import bacc, mybir, tile

    f32 = mybir.dt.float32
    bf16 = mybir.dt.bfloat16
    fp8 = mybir.dt.float8e4
    DR = mybir.MatmulPerfMode.DoubleRow

    nc = bacc.Bacc("TRN2", target_bir_lowering=False, debug=False)

    # host-packed to match the SBUF staging tiles: one fully linear
    # [P, JG, 2, 1024] block per (layer, jg) DMA
    xr = nc.dram_tensor(
        "xr", [L, JT // JG, P, JG, 2, SB_W], fp8, kind="ExternalInput"
    )
    s_out = nc.dram_tensor(
        "s_out", [len(SGROUPS), L, 16], f32, kind="ExternalOutput"
    )

    with tile.TileContext(nc) as tc:
        with (
            tc.tile_pool(name="gb", bufs=1) as gbpool,
            tc.tile_pool(name="rt", bufs=8) as rtpool,
            tc.tile_pool(name="small", bufs=2) as smallpool,
            tc.tile_pool(name="psum", bufs=1, space=bass.MemorySpace.PSUM) as psumpool,
            tc.tile_pool(name="psumS", bufs=1, space=bass.MemorySpace.PSUM) as psSpool,
        ):
            # persistent SBUF Gram store, layer-interleaved [b, c, layer]
            gbig16 = [
                gbpool.tile([P, P, L], bf16, tag=f"Gb16{r}", name=f"Gb16{r}")
                for r in range(2)
            ]
            gbig8 = gbpool.tile([P, NW2 * P, L], fp8, tag="Gb8", name="Gb8")

            # S-stage work items: per group, a list of thunks emitted a slice
            # at a time into the following layers' PE slack
            def make_s_items(g, glo, ghi):
                n = ghi          # lhsT layer span
                w = ghi - glo    # out cols
                holder = {}
                items = []

                def alloc():
                    holder["ptS"] = psSpool.tile(
                        [P, 512], f32, tag="psS", name="ptS"
                    )
                    holder["sall"] = smallpool.tile(
                        [P, 16], f32, tag="sall", name="sall"
                    )

                items.append(alloc)
                for r in range(2):
                    for c in range(P):
                        def mm0(r=r, c=c):
                            nc.tensor.matmul(
                                holder["ptS"][0:n, 0:w],
                                lhsT=gbig16[r][:, c, 0:n],
                                rhs=gbig16[r][:, c, glo:ghi],
                                start=(r == 0 and c == 0),
                                stop=(r == 1 and c == P - 1),
                            )
                        items.append(mm0)

                def copy0():
                    nc.scalar.copy(
                        holder["sall"][0:n, 0:w], holder["ptS"][0:n, 0:w]
                    )

                items.append(copy0)
                for c0 in range(0, NW2 * P, 8):
                    for x in range(4):
                        def mm1(c0=c0, x=x):
                            a4 = gbig8[:, c0 : c0 + 8, 0:n].rearrange(
                                "p (i x) l -> p i x l", i=2
                            )
                            b4 = gbig8[:, c0 : c0 + 8, glo:ghi].rearrange(
                                "p (i x) l -> p i x l", i=2
                            )
                            nc.tensor.matmul(
                                holder["ptS"][0:n, 0:w],
                                lhsT=a4[:, :, x, :],
                                rhs=b4[:, :, x, :],
                                start=(c0 == 0 and x == 0),
                                stop=(c0 == NW2 * P - 8 and x == 3),
                                perf_mode=DR,
                            )
                        items.append(mm1)

                def copy1():
                    nc.scalar.copy(
                        holder["sall"][0:n, 8 : 8 + w], holder["ptS"][0:n, 0:w]
                    )
                    nc.scalar.dma_start(
                        s_out[g, 0:n, :], holder["sall"][0:n, :]
                    )

                items.append(copy1)
                return items

            squeues = [None] * len(SGROUPS)
            for l in range(L):
                pt1 = psumpool.tile([P, 1024], f32, tag="P1", bufs=2, name="pt1")
                pt2 = psumpool.tile([P, 1536], f32, tag="P2", name="pt2")
                pts = {"P1": pt1, "P2": pt2}
                for jg in range(JT // JG):
                    rt = rtpool.tile([P, JG, 2, SB_W], fp8, tag="rt", name="rt")
                    nc.sync.dma_start(rt[:, :, :, :], xr[l, jg])
                    for jj in range(JG):
                        if no_mm:
                            continue
                        j = jg * JG + jj
                        order = (
                            RUNS if j == JT - 1 else RUNS[3:] + RUNS[:3]
                        )
                        for (tag, c0, lp, rp, w) in order:
                            nc.tensor.matmul(
                                pts[tag][:, c0 : c0 + P * w],
                                lhsT=rt[:, jj, :, lp * P : (lp + 1) * P],
                                rhs=rt[:, jj, :, rp * P : (rp + w) * P],
                                start=(j == 0),
                                stop=(j == JT - 1),
                                perf_mode=DR,
                            )
                for (tag, c0, dst, dc, w, eng) in (
                    COPIES if not (no_copy or no_mm) else []
                ):
                    src = pts[tag][:, c0 : c0 + w]
                    if dst == 2:
                        if eng == 0:
                            nc.scalar.mul(gbig8[:, dc : dc + w, l], src, 0.5)
                        else:
                            nc.vector.tensor_scalar_mul(
                                gbig8[:, dc : dc + w, l], src, 0.5
                            )
                    else:
                        nc.scalar.copy(gbig16[dst][:, :, l], src)

                # drip the pending S-stage work into this layer's PE slack
                if not (no_s or no_mm or no_copy):
                    for g, (glo, ghi, slo, shi) in enumerate(SGROUPS):
                        if l < slo or l >= shi:
                            continue
                        if squeues[g] is None:
                            squeues[g] = make_s_items(g, glo, ghi)
                        q = squeues[g]
                        take = -(-len(q) // (shi - l))
                        for _ in range(take):
                            if q:
                                q.pop(0)()

            # drain group 3 (no spread window) and any leftovers
            if not (no_s or no_mm or no_copy):
                for g, (glo, ghi, slo, shi) in enumerate(SGROUPS):
                    if squeues[g] is None:
                        squeues[g] = make_s_items(g, glo, ghi)
                    for it in squeues[g]:
                        it()
                    squeues[g] = []

    nc.compile()
    _NC_CACHE[key] = nc
    return nc


def _pack_inputs(x):
    xt_np = np.ascontiguousarray(x.transpose(0, 2, 1)).astype(ml_dtypes.float8_e4m3)
    in_maps = []
    for k in range(NCORES):
        chunk_ids = [(o + k) % 16 for o in ORDER]
        cols = np.concatenate(
            [np.arange(c * P, (c + 1) * P) for c in chunk_ids]
        )
        sel = xt_np[:, :, cols]  # [L, D, 1024]
        packed = np.ascontiguousarray(
            sel.reshape(L, JT // JG, JG, 2, P, SB_W).transpose(0, 1, 4, 2, 3, 5)
        )
        in_maps.append({"xr": packed})
    return xt_np, in_maps


def _run(activations, trace=False):
    from concourse.bass_utils import run_bass_kernel_spmd

    x = np.asarray(activations, dtype=np.float32)
    assert x.shape == (L, B, D)
    xt_np, in_maps = _pack_inputs(x)

    nc = _build()
    try:
        res = run_bass_kernel_spmd(
            nc, in_maps, core_ids=list(range(NCORES)), trace=trace
        )
    except Exception:
        # transient NRT_EXEC_UNIT_UNRECOVERABLE device states have been
        # observed to clear on the next attempt
        import time

        time.sleep(5)
        res = run_bass_kernel_spmd(
            nc, in_maps, core_ids=list(range(NCORES)), trace=trace
        )

    # accumulate the wedge S[i, j] for i < group_hi(j); mirror the rest
    S = np.zeros((L, L), dtype=np.float64)
    ghi_of = np.zeros(L, dtype=int)
    for k in range(NCORES):
        g = res.results[k]["s_out"].astype(np.float64)  # [ngrp, L, 16]
        for gi, (glo, ghi, _, _) in enumerate(SGROUPS):
            n, w = ghi, ghi - glo
            ghi_of[glo:ghi] = ghi
            S[:n, glo:ghi] += g[gi, :n, 0:w] + 8.0 * g[gi, :n, 8 : 8 + w]
    for i in range(L):
        for j in range(L):
            if i >= ghi_of[j]:
                S[i, j] = S[j, i]

    # row-sum statistics are O(L*B*D) -- computed host-side on the same
    # quantized values the device consumed
    s_star = xt_np.astype(np.float64).sum(axis=2)  # [L, D]
    xq = xt_np.astype(np.float32)                  # [L, D, B]
    rowsum = np.einsum("ldb,ld->lb", xq, s_star.astype(np.float32))
    T = np.einsum("ib,jb->ij", rowsum, rowsum, dtype=np.float64)
    u = np.einsum("ld,ld->l", s_star, s_star)
    hsic = S - (2.0 / B) * T + np.outer(u, u) / (B * B)
    norms = np.sqrt(np.diagonal(hsic))
    cka = hsic / (norms[:, None] * norms[None, :])
    return cka.astype(np.float32), res


def kernel(activations):
    cka, _ = _run(activations, trace=False)
    return cka


def run_traced(activations):
    return _run(activations, trace=True)
